# revision 6
# baseline (speedup 1.0000x reference)
"""Trainium2 Bass kernel for nn_CrossGraphNetLite (dual-GNN + gated fusion + classifier).

Strategy (8 NeuronCores, graph/data parallel, fp8 streams):
  * Host preprocesses the integer graph structure into dense coefficient
    matrices, all quantized to fp8 e4m3 (end-to-end rel err ~4e-3, tol 2e-2):
      - Layer 1 per dst-node block:  T[t, v] = sum of edge coeffs into v
        bucketed by source-node *type* t (+ self-loop + bias row). On device
        x2 = relu(ea^T T) with ea = [emb @ W1; b1] in fp16 (fp16 x fp8 matmul).
      - Layer 2 + mean-pool collapse: C[s, g] = sum of edge coeffs from src s
        (this core's block) into any node of graph g (+ self-loop), fp8.
        pool^T += h2[pair]^T C[pair] with h2 cast to fp8 and the matmuls in
        DoubleRow fp8 perf mode (2 src blocks = 2 k-tiles per instruction).
  * Partial pools are reduced with ReduceScatter(add) in two halves: the
    first RS (pairs 0..34) is issued ~70% through the stream so its fixed
    ~35us collective latency hides under the remaining C stream; the second
    RS (pairs 35..49) pipelines right behind it at the stream tail.
  * A tight per-core epilogue (gated fusion x2, LayerNorm folded into the
    classifier weights, fp16 DVE ops) finishes the core's 128 graphs.
"""

import sys

sys.path.insert(0, "/opt/trn_rl_repo")

import numpy as np
import ml_dtypes

import concourse.bacc as bacc
import concourse.bass as bass
import concourse.mybir as mybir
import concourse.tile as tile

AF = mybir.ActivationFunctionType
ALU = mybir.AluOpType
PM = mybir.MatmulPerfMode
F32 = mybir.dt.float32
F16 = mybir.dt.float16
F8 = mybir.dt.float8e4
NP_F8 = ml_dtypes.float8_e4m3


class CFG:
    def __init__(self):
        self.N = 100000
        self.E = 1250000
        self.G = 1024
        self.NCORES = 8
        self.NTA = 200
        self.NTC = 100
        self.SEM = 768
        self.NB = self.N // self.NCORES            # 12500
        self.NBP = 12800                           # padded (25 x 512)
        self.NCHUNK = 25                           # 512-node chunks
        self.NPAIR = 50                            # 256-node pairs
        self.GB = self.G // self.NCORES            # 128
        self.TPA = 256
        self.TPC = 128
        self.SEMK = 6
        self.SPLIT = 35                            # pairs 0..34 -> RS#1


def build_nc(cfg: CFG):
    nc = bacc.Bacc("TRN2", target_bir_lowering=False, debug=False,
                   enable_asserts=True, num_devices=cfg.NCORES)
    G, GB, NBP = cfg.G, cfg.GB, cfg.NBP
    RG = [list(range(cfg.NCORES))]

    def din(name, shape, dt=F32):
        return nc.dram_tensor(name, list(shape), dt, kind="ExternalInput").ap()

    T_ast = din("T_ast", [cfg.TPA, NBP], F8)
    T_cfg = din("T_cfg", [cfg.TPC, NBP], F8)
    C_ast = din("C_ast", [cfg.NPAIR, 2, 128, G], F8)
    C_cfg = din("C_cfg", [cfg.NPAIR, 2, 128, G], F8)
    embT_ast = din("embT_ast", [64, cfg.TPA])
    embT_cfg = din("embT_cfg", [64, cfg.TPC])
    astW1 = din("astW1", [64, 64])
    cfgW1 = din("cfgW1", [64, 64])
    astb1h = din("astb1h", [1, 64], F16)
    cfgb1h = din("cfgb1h", [1, 64], F16)
    astW2h = din("astW2h", [64, 64], F16)
    cfgW2h = din("cfgW2h", [64, 64], F16)
    astb2 = din("astb2", [1, 64])
    cfgb2 = din("cfgb2", [1, 64])
    cnt_ast = din("cnt_ast", [1, G])
    cnt_cfg = din("cnt_cfg", [1, G])
    Wg1h = din("Wg1h", [128, 64], F16)
    bg1c = din("bg1c", [64, 1])
    Wsemh = din("Wsemh", [cfg.SEM, 64], F16)
    bsemc = din("bsemc", [64, 1])
    semTh = din("semTh", [cfg.SEM, GB], F16)
    Wg2h = din("Wg2h", [128, 64], F16)
    bg2c = din("bg2c", [64, 1])
    Wcp = din("Wcp", [64, 2])
    bcp = din("bcp", [2, 1])
    out_ap = nc.dram_tensor("outT", [2, GB], F32, kind="ExternalOutput").ap()

    with tile.TileContext(nc) as tc:
        with (
            tc.tile_pool(name="consts", bufs=1) as consts,
            tc.tile_pool(name="x2t", bufs=1) as x2t_pool,
            tc.tile_pool(name="tstream", bufs=6) as tstream,
            tc.tile_pool(name="cstream", bufs=12) as cstream,
            tc.tile_pool(name="h2p", bufs=4) as h2p,
            tc.tile_pool(name="small", bufs=1) as small,
            tc.tile_pool(name="ps_px", bufs=2, space="PSUM") as ps_px,
            tc.tile_pool(name="ps_ph", bufs=2, space="PSUM") as ps_ph,
            tc.tile_pool(name="ps_pool", bufs=1, space="PSUM") as ps_pool,
            tc.tile_pool(name="dram", bufs=1, space="DRAM") as dram,
        ):
            # ---- critical consts on the sync (HWDGE) ring so the first
            # layer-1 matmuls can start within ~2us ----
            def load_c(eng, ap, shape, dt=F32, name=None):
                t = consts.tile(list(shape), dt, name=name or ap.tensor.name + "_sb")
                eng.dma_start(t[:], ap[:])
                return t

            embT_ast_sb = load_c(nc.sync, embT_ast, [64, cfg.TPA])
            embT_cfg_sb = load_c(nc.sync, embT_cfg, [64, cfg.TPC])
            astW1_sb = load_c(nc.sync, astW1, [64, 64])
            cfgW1_sb = load_c(nc.sync, cfgW1, [64, 64])
            astW2h_sb = load_c(nc.scalar, astW2h, [64, 64], F16)
            cfgW2h_sb = load_c(nc.scalar, cfgW2h, [64, 64], F16)
            astb2_sb = load_c(nc.scalar, astb2, [1, 64])
            cfgb2_sb = load_c(nc.scalar, cfgb2, [1, 64])
            cnt_ast_sb = load_c(nc.scalar, cnt_ast, [1, G])
            cnt_cfg_sb = load_c(nc.scalar, cnt_cfg, [1, G])

            # non-critical consts on the gpsimd (SWDGE) ring
            Wg1h_sb = load_c(nc.gpsimd, Wg1h, [128, 64], F16)
            bg1_sb = load_c(nc.gpsimd, bg1c, [64, 1])
            bsem_sb = load_c(nc.gpsimd, bsemc, [64, 1])
            Wg2h_sb = load_c(nc.gpsimd, Wg2h, [128, 64], F16)
            bg2_sb = load_c(nc.gpsimd, bg2c, [64, 1])
            Wcp_sb = load_c(nc.gpsimd, Wcp, [64, 2])
            bcp_sb = load_c(nc.gpsimd, bcp, [2, 1])
            Wsem_sb = consts.tile([128, cfg.SEMK * 64], F16, name="Wsem_sb")
            semT_sb = consts.tile([128, cfg.SEMK * GB], F16, name="semT_sb")
            for kc in range(cfg.SEMK):
                nc.gpsimd.dma_start(Wsem_sb[:, kc * 64:(kc + 1) * 64],
                                    Wsemh[kc * 128:(kc + 1) * 128, :])
                nc.gpsimd.dma_start(semT_sb[:, kc * GB:(kc + 1) * GB],
                                    semTh[kc * 128:(kc + 1) * 128, :])
            cat2 = consts.tile([128, GB], F16, name="cat2")

            # ---- ea tables: [emb @ W1; b1] in fp16 ----
            def build_ea(embT_sb, W1_sb, b1h_ap, ktiles, nt, tag):
                tiles = []
                for i in range(ktiles):
                    ps = ps_ph.tile([128, 64], F32, name=f"psea_{tag}{i}", tag="ph")
                    nc.tensor.matmul(ps[:], embT_sb[:, i * 128:(i + 1) * 128],
                                     W1_sb[:], start=True, stop=True)
                    ea = consts.tile([128, 64], F16, name=f"ea_{tag}{i}")
                    nc.vector.tensor_copy(ea[:], ps[:])
                    tiles.append(ea)
                bi, br = divmod(nt, 128)
                nc.scalar.dma_start(tiles[bi][br:br + 1, :], b1h_ap[:])
                return tiles

            ea_ast = build_ea(embT_ast_sb, astW1_sb, astb1h, 2, cfg.NTA, "a")
            ea_cfg = build_ea(embT_cfg_sb, cfgW1_sb, cfgb1h, 1, cfg.NTC, "c")

            # ---- pool PSUM accumulators; cnt*b2 is the starting matmul ----
            pool_ast = ps_pool.tile([64, G], F32, name="pool_ast")
            pool_cfg = ps_pool.tile([64, G], F32, name="pool_cfg")
            for (g0, g1) in ((0, 512), (512, 1024)):
                nc.tensor.matmul(pool_ast[:, g0:g1], astb2_sb[:],
                                 cnt_ast_sb[:, g0:g1], start=True, stop=False,
                                 skip_group_check=True)
                nc.tensor.matmul(pool_cfg[:, g0:g1], cfgb2_sb[:],
                                 cnt_cfg_sb[:, g0:g1], start=True, stop=False,
                                 skip_group_check=True)

            x2T_ast = x2t_pool.tile([64, NBP], F16, name="x2T_a", tag="x2T_a")
            x2T_cfg = x2t_pool.tile([64, NBP], F16, name="x2T_c", tag="x2T_c")

            def bstep(c):
                sl = slice(c * 512, (c + 1) * 512)
                px = ps_px.tile([64, 512], F32, name=f"pxa{c}", tag="px")
                ta0 = tstream.tile([128, 512], F8, name=f"ta0_{c}", tag="t")
                nc.sync.dma_start(ta0[:], T_ast[0:128, sl])
                ta1 = tstream.tile([128, 512], F8, name=f"ta1_{c}", tag="t")
                nc.scalar.dma_start(ta1[:], T_ast[128:256, sl])
                nc.tensor.matmul(px[:], ea_ast[0][:], ta0[:], start=True, stop=False)
                nc.tensor.matmul(px[:], ea_ast[1][:], ta1[:], start=False, stop=True)
                nc.vector.tensor_relu(x2T_ast[:, sl], px[:])
                px2 = ps_px.tile([64, 512], F32, name=f"pxc{c}", tag="px")
                tc0 = tstream.tile([128, 512], F8, name=f"tc0_{c}", tag="t")
                (nc.sync if c % 2 == 0 else nc.scalar).dma_start(tc0[:], T_cfg[:, sl])
                nc.tensor.matmul(px2[:], ea_cfg[0][:], tc0[:], start=True, stop=True)
                nc.vector.tensor_relu(x2T_cfg[:, sl], px2[:])

            def h2pair(s2):
                # h2 (fp8) for src blocks 2*s2, 2*s2+1, both graph types
                ph = ps_ph.tile([128, 256], F32, name=f"ph{s2}", tag="ph")
                for j in range(2):
                    blk = 2 * s2 + j
                    nc.tensor.matmul(ph[:, j * 64:(j + 1) * 64],
                                     x2T_ast[:, blk * 128:(blk + 1) * 128],
                                     astW2h_sb[:], start=True, stop=True)
                for j in range(2):
                    blk = 2 * s2 + j
                    nc.tensor.matmul(ph[:, 128 + j * 64:128 + (j + 1) * 64],
                                     x2T_cfg[:, blk * 128:(blk + 1) * 128],
                                     cfgW2h_sb[:], start=True, stop=True)
                h2q = h2p.tile([128, 256], F8, name=f"h2_{s2}", tag="h2")
                nc.vector.tensor_copy(h2q[:], ph[:])
                return h2q

            def poolstep(s2, h2q):
                start = (s2 == cfg.SPLIT)
                stop = s2 in (cfg.SPLIT - 1, cfg.NPAIR - 1)
                for (C_ap, joff, pool_ps, tag) in ((C_ast, 0, pool_ast, "a"),
                                                   (C_cfg, 128, pool_cfg, "c")):
                    ct = cstream.tile([128, 2048], F8, name=f"c{tag}{s2}", tag="c")
                    nc.sync.dma_start(ct[:, 0:1024], C_ap[s2, 0])
                    nc.scalar.dma_start(ct[:, 1024:2048], C_ap[s2, 1])
                    lhsT = h2q[:, joff:joff + 128].rearrange(
                        "p (two m) -> p two m", two=2)
                    rhs3 = ct[:].rearrange("p (two g) -> p two g", two=2)
                    for (g0, g1) in ((0, 512), (512, 1024)):
                        nc.tensor.matmul(pool_ps[:, g0:g1], lhsT, rhs3[:, :, g0:g1],
                                         start=start, stop=stop,
                                         perf_mode=PM.DoubleRow,
                                         skip_group_check=True)

            def flush(idx):
                pA = small.tile([64, G], F16, name=f"pA{idx}")
                pC = small.tile([64, G], F16, name=f"pC{idx}")
                nc.vector.tensor_copy(pA[:], pool_ast[:])
                nc.vector.tensor_copy(pC[:], pool_cfg[:])
                rs_in = dram.tile([cfg.NCORES, 128, GB], F16, name=f"rsin{idx}")
                rs_out = dram.tile([128, GB], F16, name=f"rsout{idx}")
                nc.sync.dma_start(
                    rs_in[:, 0:64, :].rearrange("j p d -> p j d"),
                    pA[:].rearrange("p (j d) -> p j d", j=cfg.NCORES))
                nc.scalar.dma_start(
                    rs_in[:, 64:128, :].rearrange("j p d -> p j d"),
                    pC[:].rearrange("p (j d) -> p j d", j=cfg.NCORES))
                nc.gpsimd.collective_compute(
                    "ReduceScatter", ALU.add, replica_groups=RG,
                    ins=[rs_in.opt()], outs=[rs_out.opt()])
                return rs_out

            # ---- fused streaming loop ----
            rs_outs = []
            bstep(0)
            for c in range(cfg.NCHUNK):
                for s2 in (2 * c, 2 * c + 1):
                    h2q = h2pair(s2)
                    if s2 % 2 == 0 and c + 1 < cfg.NCHUNK:
                        bstep(c + 1)
                    poolstep(s2, h2q)
                    if s2 == cfg.SPLIT - 1:
                        rs_outs.append(flush(0))
                        # semantic branch: overlaps the stream tail
                        pssem = ps_px.tile([64, GB], F32, name="pssem", tag="px")
                        for kc in range(cfg.SEMK):
                            nc.tensor.matmul(pssem[:],
                                             Wsem_sb[:, kc * 64:(kc + 1) * 64],
                                             semT_sb[:, kc * GB:(kc + 1) * GB],
                                             start=(kc == 0),
                                             stop=(kc == cfg.SEMK - 1))
                        hsem = small.tile([64, GB], F16, name="hsem")
                        nc.scalar.activation(hsem[:], pssem[:], AF.Relu,
                                             bias=bsem_sb[:])
                        nc.gpsimd.dma_start(cat2[64:128, :], hsem[:])
            rs_outs.append(flush(1))

            # ---- epilogue for this core's GB graphs ----
            r0, r1 = rs_outs
            tmpA = small.tile([128, GB], F16, name="tmpA")
            tmpB = small.tile([128, GB], F16, name="tmpB")
            nc.sync.dma_start(tmpA[:], r0[:])
            nc.scalar.dma_start(tmpB[:], r1[:])
            cat = small.tile([128, GB], F16, name="cat")
            nc.vector.tensor_add(cat[:], tmpA[:], tmpB[:])
            hAC = small.tile([64, 2 * GB], F16, name="hAC")
            nc.sync.dma_start(hAC[:, 0:GB], cat[0:64, :])
            nc.scalar.dma_start(hAC[:, GB:2 * GB], cat[64:128, :])

            # gated fuse 1: hs = hC + g1*(hA - hC)
            psg1 = ps_px.tile([64, GB], F32, name="psg1", tag="px")
            nc.tensor.matmul(psg1[:], Wg1h_sb[:], cat[:], start=True, stop=True)
            g1 = small.tile([64, GB], F16, name="g1")
            nc.scalar.activation(g1[:], psg1[:], AF.Sigmoid, bias=bg1_sb[:])
            d1 = small.tile([64, GB], F16, name="d1")
            nc.vector.tensor_sub(d1[:], hAC[:, 0:GB], hAC[:, GB:2 * GB])
            t1 = small.tile([64, GB], F16, name="t1")
            nc.vector.tensor_mul(t1[:], g1[:], d1[:])
            hs = small.tile([64, GB], F16, name="hs")
            nc.vector.tensor_add(hs[:], hAC[:, GB:2 * GB], t1[:])
            nc.sync.dma_start(cat2[0:64, :], hs[:])

            # gated fuse 2 with the semantic branch
            psg2 = ps_px.tile([64, GB], F32, name="psg2", tag="px")
            nc.tensor.matmul(psg2[:], Wg2h_sb[:], cat2[:], start=True, stop=True)
            g2 = small.tile([64, GB], F16, name="g2")
            nc.scalar.activation(g2[:], psg2[:], AF.Sigmoid, bias=bg2_sb[:])
            d2 = small.tile([64, GB], F16, name="d2")
            nc.vector.tensor_sub(d2[:], hs[:], hsem[:])
            t2 = small.tile([64, GB], F16, name="t2")
            nc.vector.tensor_mul(t2[:], g2[:], d2[:])
            hh = small.tile([64, 2 * GB], F32, name="hh")
            nc.vector.tensor_add(hh[:, 0:GB], hsem[:], t2[:])
            nc.vector.tensor_mul(hh[:, GB:2 * GB], hh[:, 0:GB], hh[:, 0:GB])

            # LayerNorm folded into classifier: out = ((h-mu)*rstd) @ Wc' + bc'
            ones64 = small.tile([64, 1], F32, name="ones64")
            nc.vector.memset(ones64[:], 1.0 / 64.0)
            ones1 = small.tile([1, 64], F32, name="ones1")
            nc.vector.memset(ones1[:], 1.0)
            ps2 = ps_ph.tile([1, 2 * GB], F32, name="ps2", tag="ph")
            nc.tensor.matmul(ps2[:], ones64[:], hh[:], start=True, stop=True)
            row2 = small.tile([1, 2 * GB], F32, name="row2")
            nc.vector.tensor_copy(row2[:], ps2[:])
            mu2 = small.tile([1, GB], F32, name="mu2")
            nc.vector.tensor_mul(mu2[:], row2[:, 0:GB], row2[:, 0:GB])
            var = small.tile([1, GB], F32, name="var")
            nc.vector.tensor_sub(var[:], row2[:, GB:2 * GB], mu2[:])
            eps = small.tile([1, 1], F32, name="eps")
            nc.vector.memset(eps[:], 1e-5)
            sd = small.tile([1, GB], F32, name="sd")
            nc.scalar.activation(sd[:], var[:], AF.Sqrt, bias=eps[:])
            brow = small.tile([1, 2 * GB], F32, name="brow")
            nc.vector.reciprocal(brow[:, 0:GB], sd[:])
            nc.vector.tensor_mul(brow[:, GB:2 * GB], row2[:, 0:GB], brow[:, 0:GB])
            psb = ps_px.tile([64, 2 * GB], F32, name="psb", tag="px")
            nc.tensor.matmul(psb[:], ones1[:], brow[:], start=True, stop=True)
            z = small.tile([64, GB], F32, name="z")
            nc.vector.tensor_mul(z[:], hh[:, 0:GB], psb[:, 0:GB])
            nc.vector.tensor_sub(z[:], z[:], psb[:, GB:2 * GB])
            psout = ps_ph.tile([2, GB], F32, name="psout", tag="ph")
            nc.tensor.matmul(psout[:], Wcp_sb[:], z[:], start=True, stop=True)
            outT_sb = small.tile([2, GB], F32, name="outT_sb")
            nc.vector.tensor_scalar_add(outT_sb[:], psout[:], bcp_sb[:])
            nc.sync.dma_start(out_ap[:], outT_sb[:])

    nc.compile()
    return nc


# ---------------------------------------------------------------------------
# host-side preprocessing
# ---------------------------------------------------------------------------

def preprocess(inputs: dict, cfg: CFG):
    N, G, NB, NBP, GB = cfg.N, cfg.G, cfg.NB, cfg.NBP, cfg.GB

    def graph_structs(edge, types, batch, tp, nt):
        src = np.asarray(edge[0], np.int64)
        dst = np.asarray(edge[1], np.int64)
        types = np.asarray(types, np.int64)
        batch = np.asarray(batch, np.int64)
        deg = (np.bincount(dst, minlength=N) + 1.0).astype(np.float32)
        dinv = (1.0 / np.sqrt(deg)).astype(np.float32)
        coeff = (dinv[src] * dinv[dst]).astype(np.float32)
        selfc = (dinv * dinv).astype(np.float32)
        t_src = types[src]
        g_dst = batch[dst]
        counts = np.bincount(batch, minlength=G).astype(np.float32)
        Ts, Cs, cnts = [], [], []
        for k in range(cfg.NCORES):
            lo, hi = k * NB, (k + 1) * NB
            m = (dst >= lo) & (dst < hi)
            flat = t_src[m] * NBP + (dst[m] - lo)
            T = np.bincount(flat, weights=coeff[m].astype(np.float64),
                            minlength=tp * NBP)
            blk = np.arange(lo, hi)
            flat_self = types[blk] * NBP + (blk - lo)
            T += np.bincount(flat_self, weights=selfc[blk].astype(np.float64),
                             minlength=tp * NBP)
            T = T.reshape(tp, NBP).astype(np.float32)
            T[nt, 0:NB] = 1.0   # bias row
            Ts.append(T.astype(NP_F8))
            m2 = (src >= lo) & (src < hi)
            flat2 = (src[m2] - lo) * G + g_dst[m2]
            C = np.bincount(flat2, weights=coeff[m2].astype(np.float64),
                            minlength=NBP * G)
            flat2s = (blk - lo) * G + batch[blk]
            C += np.bincount(flat2s, weights=selfc[blk].astype(np.float64),
                             minlength=NBP * G)
            C = C.reshape(NBP, G).astype(np.float32).astype(NP_F8)
            Cs.append(np.ascontiguousarray(C.reshape(cfg.NPAIR, 2, 128, G)))
            cm = np.zeros((1, G), np.float32)
            cm[0, k * GB:(k + 1) * GB] = counts[k * GB:(k + 1) * GB]
            cnts.append(cm)
        return Ts, Cs, cnts

    Ta, Ca, cnta = graph_structs(inputs["ast_edge"], inputs["ast_type"],
                                 inputs["ast_batch"], cfg.TPA, cfg.NTA)
    Tc, Cc, cntc = graph_structs(inputs["cfg_edge"], inputs["cfg_type"],
                                 inputs["cfg_batch"], cfg.TPC, cfg.NTC)

    f32 = lambda x: np.ascontiguousarray(np.asarray(x, np.float32))
    f16 = lambda x: np.ascontiguousarray(np.asarray(x, np.float32).astype(np.float16))
    embT_ast = np.zeros((64, cfg.TPA), np.float32)
    embT_ast[:, 0:cfg.NTA] = f32(inputs["ast_emb"]).T
    embT_cfg = np.zeros((64, cfg.TPC), np.float32)
    embT_cfg[:, 0:cfg.NTC] = f32(inputs["cfg_emb"]).T
    semT = f32(inputs["struct_sem"]).T.copy()  # [SEM, G]

    ln_g = f32(inputs["ln_g"])
    ln_b = f32(inputs["ln_b"])
    Wc = f32(inputs["Wc"])
    Wcp = np.ascontiguousarray(ln_g[:, None] * Wc)
    bcp = (ln_b @ Wc + f32(inputs["bc"])).reshape(2, 1)

    shared = {
        "embT_ast": embT_ast, "embT_cfg": embT_cfg,
        "astW1": f32(inputs["ast_W1"]), "cfgW1": f32(inputs["cfg_W1"]),
        "astb1h": f16(inputs["ast_b1"]).reshape(1, 64),
        "cfgb1h": f16(inputs["cfg_b1"]).reshape(1, 64),
        "astW2h": f16(inputs["ast_W2"]), "cfgW2h": f16(inputs["cfg_W2"]),
        "astb2": f32(inputs["ast_b2"]).reshape(1, 64),
        "cfgb2": f32(inputs["cfg_b2"]).reshape(1, 64),
        "Wg1h": f16(inputs["Wg1"]), "bg1c": f32(inputs["bg1"]).reshape(64, 1),
        "Wsemh": f16(inputs["Wsem"]), "bsemc": f32(inputs["bsem"]).reshape(64, 1),
        "Wg2h": f16(inputs["Wg2"]), "bg2c": f32(inputs["bg2"]).reshape(64, 1),
        "Wcp": Wcp, "bcp": np.ascontiguousarray(bcp),
    }
    in_maps = []
    for k in range(cfg.NCORES):
        m = dict(shared)
        m["T_ast"] = Ta[k]
        m["T_cfg"] = Tc[k]
        m["C_ast"] = Ca[k]
        m["C_cfg"] = Cc[k]
        m["cnt_ast"] = cnta[k]
        m["cnt_cfg"] = cntc[k]
        m["semTh"] = np.ascontiguousarray(
            semT[:, k * GB:(k + 1) * GB].astype(np.float16))
        in_maps.append(m)
    return in_maps


def postprocess(results, cfg: CFG):
    outs = [np.asarray(results[k]["outT"]) for k in range(cfg.NCORES)]
    return np.concatenate(outs, axis=1).T.copy()  # [G, 2]


_CACHED = {}


def kernel(**inputs):
    from concourse.bass_utils import run_bass_kernel_spmd
    cfg = CFG()
    if "nc" not in _CACHED:
        _CACHED["nc"] = build_nc(cfg)
    in_maps = preprocess(inputs, cfg)
    res = run_bass_kernel_spmd(_CACHED["nc"], in_maps,
                               core_ids=list(range(cfg.NCORES)))
    return postprocess(res.results, cfg)


# revision 13
# speedup vs baseline: 1.2460x; 1.2460x over previous
"""Trainium2 Bass kernel for nn_CrossGraphNetLite (dual-GNN + gated fusion + classifier).

Strategy (8 NeuronCores, graph/data parallel, fp8 streams):
  * Host preprocesses the integer graph structure into dense coefficient
    matrices, all quantized to fp8 e4m3 (end-to-end rel err ~4e-3, tol 2e-2):
      - Layer 1 per dst-node block:  T[t, v] = sum of edge coeffs into v
        bucketed by source-node *type* t (+ self-loop + bias row). On device
        x2 = relu(ea^T T) with ea = [emb @ W1; b1] in fp16 (fp16 x fp8 matmul).
      - Layer 2 + mean-pool collapse: C[s, g] = sum of edge coeffs from src s
        (this core's block) into any node of graph g (+ self-loop), fp8.
        pool^T += h2[pair]^T C[pair] with h2 cast to fp8 and the matmuls in
        DoubleRow fp8 perf mode (2 src blocks = 2 k-tiles per instruction).
  * Partial pools are reduced with ReduceScatter(add) in two halves: the
    first RS (pairs 0..34) is issued ~70% through the stream so its fixed
    ~35us collective latency hides under the remaining C stream; the second
    RS (pairs 35..49) pipelines right behind it at the stream tail.
  * A tight per-core epilogue (gated fusion x2, LayerNorm folded into the
    classifier weights, fp16 DVE ops) finishes the core's 128 graphs.
"""

import sys

sys.path.insert(0, "/opt/trn_rl_repo")

import numpy as np
import ml_dtypes

import concourse.bacc as bacc
import concourse.bass as bass
import concourse.mybir as mybir
import concourse.tile as tile

AF = mybir.ActivationFunctionType
ALU = mybir.AluOpType
PM = mybir.MatmulPerfMode
F32 = mybir.dt.float32
F16 = mybir.dt.float16
F8 = mybir.dt.float8e4
NP_F8 = ml_dtypes.float8_e4m3


class CFG:
    def __init__(self):
        self.N = 100000
        self.E = 1250000
        self.G = 1024
        self.NCORES = 8
        self.NTA = 200
        self.NTC = 100
        self.SEM = 768
        self.NB = self.N // self.NCORES            # 12500
        self.NBP = 12800                           # padded (25 x 512)
        self.NCHUNK = 25                           # 512-node chunks
        self.NPAIR = 50                            # 256-node pairs
        self.GB = self.G // self.NCORES            # 128
        self.TPA = 256
        self.TPC = 128
        self.SEMK = 6
        self.SPLIT = 35                            # pairs 0..34 -> RS#1


def build_nc(cfg: CFG):
    nc = bacc.Bacc("TRN2", target_bir_lowering=False, debug=False,
                   enable_asserts=True, num_devices=cfg.NCORES)
    G, GB, NBP = cfg.G, cfg.GB, cfg.NBP
    RG = [list(range(cfg.NCORES))]

    def din(name, shape, dt=F32):
        return nc.dram_tensor(name, list(shape), dt, kind="ExternalInput").ap()

    T_ast = din("T_ast", [cfg.TPA, NBP], F8)
    T_cfg = din("T_cfg", [cfg.TPC, NBP], F8)
    C_ast = din("C_ast", [cfg.NPAIR, 2, 128, G], F8)
    C_cfg = din("C_cfg", [cfg.NPAIR, 2, 128, G], F8)
    embT_ast = din("embT_ast", [64, cfg.TPA])
    embT_cfg = din("embT_cfg", [64, cfg.TPC])
    astW1 = din("astW1", [64, 64])
    cfgW1 = din("cfgW1", [64, 64])
    astb1h = din("astb1h", [1, 64], F16)
    cfgb1h = din("cfgb1h", [1, 64], F16)
    astW2h = din("astW2h", [64, 64], F16)
    cfgW2h = din("cfgW2h", [64, 64], F16)
    astb2 = din("astb2", [1, 64])
    cfgb2 = din("cfgb2", [1, 64])
    cnt_ast = din("cnt_ast", [1, G])
    cnt_cfg = din("cnt_cfg", [1, G])
    Wg1h = din("Wg1h", [128, 64], F16)
    bg1c = din("bg1c", [64, 1])
    Wsemh = din("Wsemh", [cfg.SEM, 64], F16)
    bsemc = din("bsemc", [64, 1])
    semTh = din("semTh", [cfg.SEM, GB], F16)
    Wg2h = din("Wg2h", [128, 64], F16)
    bg2c = din("bg2c", [64, 1])
    Wcp = din("Wcp", [64, 2])
    bcp = din("bcp", [2, 1])
    out_ap = nc.dram_tensor("outT", [2, GB], F32, kind="ExternalOutput").ap()

    with tile.TileContext(nc) as tc:
        with (
            tc.tile_pool(name="consts", bufs=1) as consts,
            tc.tile_pool(name="x2t", bufs=1) as x2t_pool,
            tc.tile_pool(name="tstream", bufs=6) as tstream,
            tc.tile_pool(name="cstream", bufs=12) as cstream,
            tc.tile_pool(name="h2p", bufs=4) as h2p,
            tc.tile_pool(name="small", bufs=1) as small,
            tc.tile_pool(name="ps_px", bufs=2, space="PSUM") as ps_px,
            tc.tile_pool(name="ps_ph", bufs=2, space="PSUM") as ps_ph,
            tc.tile_pool(name="ps_pool", bufs=1, space="PSUM") as ps_pool,
            tc.tile_pool(name="dram", bufs=1, space="DRAM") as dram,
        ):
            # ---- critical consts on the sync (HWDGE) ring so the first
            # layer-1 matmuls can start within ~2us ----
            def load_c(eng, ap, shape, dt=F32, name=None):
                t = consts.tile(list(shape), dt, name=name or ap.tensor.name + "_sb")
                eng.dma_start(t[:], ap[:])
                return t

            embT_ast_sb = load_c(nc.sync, embT_ast, [64, cfg.TPA])
            embT_cfg_sb = load_c(nc.sync, embT_cfg, [64, cfg.TPC])
            astW1_sb = load_c(nc.sync, astW1, [64, 64])
            cfgW1_sb = load_c(nc.sync, cfgW1, [64, 64])
            astW2h_sb = load_c(nc.scalar, astW2h, [64, 64], F16)
            cfgW2h_sb = load_c(nc.scalar, cfgW2h, [64, 64], F16)
            astb2_sb = load_c(nc.scalar, astb2, [1, 64])
            cfgb2_sb = load_c(nc.scalar, cfgb2, [1, 64])
            cnt_ast_sb = load_c(nc.scalar, cnt_ast, [1, G])
            cnt_cfg_sb = load_c(nc.scalar, cnt_cfg, [1, G])

            # non-critical consts on the gpsimd (SWDGE) ring
            Wg1h_sb = load_c(nc.gpsimd, Wg1h, [128, 64], F16)
            bg1_sb = load_c(nc.gpsimd, bg1c, [64, 1])
            bsem_sb = load_c(nc.gpsimd, bsemc, [64, 1])
            Wg2h_sb = load_c(nc.gpsimd, Wg2h, [128, 64], F16)
            bg2_sb = load_c(nc.gpsimd, bg2c, [64, 1])
            Wcp_sb = load_c(nc.gpsimd, Wcp, [64, 2])
            bcp_sb = load_c(nc.gpsimd, bcp, [2, 1])
            Wsem_sb = consts.tile([128, cfg.SEMK * 64], F16, name="Wsem_sb")
            semT_sb = consts.tile([128, cfg.SEMK * GB], F16, name="semT_sb")
            for kc in range(cfg.SEMK):
                nc.gpsimd.dma_start(Wsem_sb[:, kc * 64:(kc + 1) * 64],
                                    Wsemh[kc * 128:(kc + 1) * 128, :])
                nc.gpsimd.dma_start(semT_sb[:, kc * GB:(kc + 1) * GB],
                                    semTh[kc * 128:(kc + 1) * 128, :])
            cat2 = consts.tile([128, GB], F16, name="cat2")

            # ---- ea tables: [emb @ W1; b1] in fp16 ----
            def build_ea(embT_sb, W1_sb, b1h_ap, ktiles, nt, tag):
                tiles = []
                for i in range(ktiles):
                    ps = ps_ph.tile([128, 64], F32, name=f"psea_{tag}{i}", tag="ph")
                    nc.tensor.matmul(ps[:], embT_sb[:, i * 128:(i + 1) * 128],
                                     W1_sb[:], start=True, stop=True)
                    ea = consts.tile([128, 64], F16, name=f"ea_{tag}{i}")
                    nc.vector.tensor_copy(ea[:], ps[:])
                    tiles.append(ea)
                bi, br = divmod(nt, 128)
                nc.scalar.dma_start(tiles[bi][br:br + 1, :], b1h_ap[:])
                return tiles

            ea_ast = build_ea(embT_ast_sb, astW1_sb, astb1h, 2, cfg.NTA, "a")
            ea_cfg = build_ea(embT_cfg_sb, cfgW1_sb, cfgb1h, 1, cfg.NTC, "c")

            # ---- pool PSUM accumulators; cnt*b2 is the starting matmul ----
            pool_ast = ps_pool.tile([64, G], F32, name="pool_ast")
            pool_cfg = ps_pool.tile([64, G], F32, name="pool_cfg")
            for (g0, g1) in ((0, 512), (512, 1024)):
                nc.tensor.matmul(pool_ast[:, g0:g1], astb2_sb[:],
                                 cnt_ast_sb[:, g0:g1], start=True, stop=False,
                                 skip_group_check=True)
                nc.tensor.matmul(pool_cfg[:, g0:g1], cfgb2_sb[:],
                                 cnt_cfg_sb[:, g0:g1], start=True, stop=False,
                                 skip_group_check=True)

            x2T_ast = x2t_pool.tile([64, NBP], F16, name="x2T_a", tag="x2T_a")
            x2T_cfg = x2t_pool.tile([64, NBP], F16, name="x2T_c", tag="x2T_c")

            def bstep(c):
                sl = slice(c * 512, (c + 1) * 512)
                px = ps_px.tile([64, 512], F32, name=f"pxa{c}", tag="px")
                ta0 = tstream.tile([128, 512], F8, name=f"ta0_{c}", tag="t")
                nc.gpsimd.dma_start(ta0[:], T_ast[0:128, sl])
                ta1 = tstream.tile([128, 512], F8, name=f"ta1_{c}", tag="t")
                nc.gpsimd.dma_start(ta1[:], T_ast[128:256, sl])
                nc.tensor.matmul(px[:], ea_ast[0][:], ta0[:], start=True, stop=False)
                nc.tensor.matmul(px[:], ea_ast[1][:], ta1[:], start=False, stop=True)
                nc.scalar.activation(x2T_ast[:, sl], px[:], AF.Relu)
                px2 = ps_px.tile([64, 512], F32, name=f"pxc{c}", tag="px")
                tc0 = tstream.tile([128, 512], F8, name=f"tc0_{c}", tag="t")
                nc.gpsimd.dma_start(tc0[:], T_cfg[:, sl])
                nc.tensor.matmul(px2[:], ea_cfg[0][:], tc0[:], start=True, stop=True)
                nc.scalar.activation(x2T_cfg[:, sl], px2[:], AF.Relu)

            def h2pair(s2):
                # h2 (fp8) for src blocks 2*s2, 2*s2+1, both graph types
                ph = ps_ph.tile([128, 256], F32, name=f"ph{s2}", tag="ph")
                for j in range(2):
                    blk = 2 * s2 + j
                    nc.tensor.matmul(ph[:, j * 64:(j + 1) * 64],
                                     x2T_ast[:, blk * 128:(blk + 1) * 128],
                                     astW2h_sb[:], start=True, stop=True)
                for j in range(2):
                    blk = 2 * s2 + j
                    nc.tensor.matmul(ph[:, 128 + j * 64:128 + (j + 1) * 64],
                                     x2T_cfg[:, blk * 128:(blk + 1) * 128],
                                     cfgW2h_sb[:], start=True, stop=True)
                h2q = h2p.tile([128, 256], F8, name=f"h2_{s2}", tag="h2")
                nc.vector.tensor_copy(h2q[:], ph[:])
                return h2q

            def poolstep(s2, h2q):
                stop = (s2 == cfg.NPAIR - 1)
                for (C_ap, joff, pool_ps, tag) in ((C_ast, 0, pool_ast, "a"),
                                                   (C_cfg, 128, pool_cfg, "c")):
                    ct = cstream.tile([128, 2048], F8, name=f"c{tag}{s2}", tag="c")
                    nc.sync.dma_start(ct[:, 0:1024], C_ap[s2, 0])
                    nc.scalar.dma_start(ct[:, 1024:2048], C_ap[s2, 1])
                    lhsT = h2q[:, joff:joff + 128].rearrange(
                        "p (two m) -> p two m", two=2)
                    rhs3 = ct[:].rearrange("p (two g) -> p two g", two=2)
                    for (g0, g1) in ((0, 512), (512, 1024)):
                        nc.tensor.matmul(pool_ps[:, g0:g1], lhsT, rhs3[:, :, g0:g1],
                                         start=False, stop=stop,
                                         perf_mode=PM.DoubleRow,
                                         skip_group_check=True)

            # warm up the CC RDH stream so the tail ReduceScatter starts hot
            warm_in = dram.tile([cfg.NCORES, 1, 8], F16, name="warm_in")
            warm_out = dram.tile([1, 8], F16, name="warm_out")
            wz = small.tile([1, 64], F16, name="wz")
            nc.vector.memset(wz[:], 0.0)
            nc.sync.dma_start(
                warm_in[:, :, :].rearrange("j p d -> p j d"),
                wz[:].rearrange("p (j d) -> p j d", j=cfg.NCORES))
            nc.gpsimd.collective_compute(
                "ReduceScatter", ALU.add, replica_groups=RG,
                ins=[warm_in.opt()], outs=[warm_out.opt()])

            # epilogue constants, issued early so they execute during the ramp
            ones64 = small.tile([64, 1], F32, name="ones64")
            nc.vector.memset(ones64[:], 1.0 / 64.0)
            ones1 = small.tile([1, 64], F32, name="ones1")
            nc.vector.memset(ones1[:], 1.0)
            eps = small.tile([1, 1], F32, name="eps")
            nc.vector.memset(eps[:], 1e-5)

            # ---- fused streaming loop ----
            bstep(0)
            for c in range(cfg.NCHUNK):
                for s2 in (2 * c, 2 * c + 1):
                    h2q = h2pair(s2)
                    if s2 % 2 == 0 and c + 1 < cfg.NCHUNK:
                        bstep(c + 1)
                    poolstep(s2, h2q)
                    if s2 == cfg.SPLIT - 1:
                        # semantic branch: overlaps the stream tail
                        pssem = ps_px.tile([64, GB], F32, name="pssem", tag="px")
                        for kc in range(cfg.SEMK):
                            nc.tensor.matmul(pssem[:],
                                             Wsem_sb[:, kc * 64:(kc + 1) * 64],
                                             semT_sb[:, kc * GB:(kc + 1) * GB],
                                             start=(kc == 0),
                                             stop=(kc == cfg.SEMK - 1))
                        hsem = small.tile([64, GB], F16, name="hsem")
                        nc.scalar.activation(hsem[:], pssem[:], AF.Relu,
                                             bias=bsem_sb[:])
                        nc.gpsimd.dma_start(cat2[64:128, :], hsem[:])

            # ---- drain: evacuate pools, ReduceScatter across the 8 cores ----
            pA = small.tile([64, G], F16, name="pA")
            pC = small.tile([64, G], F16, name="pC")
            nc.vector.tensor_copy(pA[:], pool_ast[:])
            nc.vector.tensor_copy(pC[:], pool_cfg[:])
            rs_in = dram.tile([cfg.NCORES, 128, GB], F16, name="rsin")
            rs_out = dram.tile([128, GB], F16, name="rsout")
            nc.sync.dma_start(
                rs_in[:, 0:64, :].rearrange("j p d -> p j d"),
                pA[:].rearrange("p (j d) -> p j d", j=cfg.NCORES))
            nc.scalar.dma_start(
                rs_in[:, 64:128, :].rearrange("j p d -> p j d"),
                pC[:].rearrange("p (j d) -> p j d", j=cfg.NCORES))
            nc.gpsimd.collective_compute(
                "ReduceScatter", ALU.add, replica_groups=RG,
                ins=[rs_in.opt()], outs=[rs_out.opt()])

            # ---- epilogue for this core's GB graphs ----
            cat = small.tile([128, GB], F16, name="cat")
            nc.sync.dma_start(cat[:], rs_out[:])
            hAC = small.tile([64, 2 * GB], F16, name="hAC")
            nc.scalar.dma_start(hAC[:, 0:GB], rs_out[0:64, :])
            nc.scalar.dma_start(hAC[:, GB:2 * GB], rs_out[64:128, :])

            # gated fuse 1: hs = hC + g1*(hA - hC)
            psg1 = ps_px.tile([64, GB], F32, name="psg1", tag="px")
            nc.tensor.matmul(psg1[:], Wg1h_sb[:], cat[:], start=True, stop=True)
            g1 = small.tile([64, GB], F16, name="g1")
            nc.scalar.activation(g1[:], psg1[:], AF.Sigmoid, bias=bg1_sb[:])
            d1 = small.tile([64, GB], F16, name="d1")
            nc.vector.tensor_sub(d1[:], hAC[:, 0:GB], hAC[:, GB:2 * GB])
            t1 = small.tile([64, GB], F16, name="t1")
            nc.vector.tensor_mul(t1[:], g1[:], d1[:])
            hs = small.tile([64, GB], F16, name="hs")
            nc.vector.tensor_add(hs[:], hAC[:, GB:2 * GB], t1[:])
            nc.sync.dma_start(cat2[0:64, :], hs[:])

            # gated fuse 2 with the semantic branch
            psg2 = ps_px.tile([64, GB], F32, name="psg2", tag="px")
            nc.tensor.matmul(psg2[:], Wg2h_sb[:], cat2[:], start=True, stop=True)
            g2 = small.tile([64, GB], F16, name="g2")
            nc.scalar.activation(g2[:], psg2[:], AF.Sigmoid, bias=bg2_sb[:])
            d2 = small.tile([64, GB], F16, name="d2")
            nc.vector.tensor_sub(d2[:], hs[:], hsem[:])
            t2 = small.tile([64, GB], F16, name="t2")
            nc.vector.tensor_mul(t2[:], g2[:], d2[:])
            hh = small.tile([64, 2 * GB], F32, name="hh")
            nc.vector.tensor_add(hh[:, 0:GB], hsem[:], t2[:])
            nc.vector.tensor_mul(hh[:, GB:2 * GB], hh[:, 0:GB], hh[:, 0:GB])

            # LayerNorm folded into classifier: out = ((h-mu)*rstd) @ Wc' + bc'
            ps2 = ps_ph.tile([1, 2 * GB], F32, name="ps2", tag="ph")
            nc.tensor.matmul(ps2[:], ones64[:], hh[:], start=True, stop=True)
            row2 = small.tile([1, 2 * GB], F32, name="row2")
            nc.vector.tensor_copy(row2[:], ps2[:])
            mu2 = small.tile([1, GB], F32, name="mu2")
            nc.vector.tensor_mul(mu2[:], row2[:, 0:GB], row2[:, 0:GB])
            var = small.tile([1, GB], F32, name="var")
            nc.vector.tensor_sub(var[:], row2[:, GB:2 * GB], mu2[:])
            sd = small.tile([1, GB], F32, name="sd")
            nc.scalar.activation(sd[:], var[:], AF.Sqrt, bias=eps[:])
            brow = small.tile([1, 2 * GB], F32, name="brow")
            nc.vector.reciprocal(brow[:, 0:GB], sd[:])
            nc.vector.tensor_mul(brow[:, GB:2 * GB], row2[:, 0:GB], brow[:, 0:GB])
            psb = ps_px.tile([64, 2 * GB], F32, name="psb", tag="px")
            nc.tensor.matmul(psb[:], ones1[:], brow[:], start=True, stop=True)
            z = small.tile([64, GB], F32, name="z")
            nc.vector.tensor_mul(z[:], hh[:, 0:GB], psb[:, 0:GB])
            nc.vector.tensor_sub(z[:], z[:], psb[:, GB:2 * GB])
            psout = ps_ph.tile([2, GB], F32, name="psout", tag="ph")
            nc.tensor.matmul(psout[:], Wcp_sb[:], z[:], start=True, stop=True)
            outT_sb = small.tile([2, GB], F32, name="outT_sb")
            nc.vector.tensor_scalar_add(outT_sb[:], psout[:], bcp_sb[:])
            nc.sync.dma_start(out_ap[:], outT_sb[:])

    nc.compile()
    return nc


# ---------------------------------------------------------------------------
# host-side preprocessing
# ---------------------------------------------------------------------------

def preprocess(inputs: dict, cfg: CFG):
    N, G, NB, NBP, GB = cfg.N, cfg.G, cfg.NB, cfg.NBP, cfg.GB

    def graph_structs(edge, types, batch, tp, nt):
        src = np.asarray(edge[0], np.int64)
        dst = np.asarray(edge[1], np.int64)
        types = np.asarray(types, np.int64)
        batch = np.asarray(batch, np.int64)
        deg = (np.bincount(dst, minlength=N) + 1.0).astype(np.float32)
        dinv = (1.0 / np.sqrt(deg)).astype(np.float32)
        coeff = (dinv[src] * dinv[dst]).astype(np.float32)
        selfc = (dinv * dinv).astype(np.float32)
        t_src = types[src]
        g_dst = batch[dst]
        counts = np.bincount(batch, minlength=G).astype(np.float32)
        Ts, Cs, cnts = [], [], []
        for k in range(cfg.NCORES):
            lo, hi = k * NB, (k + 1) * NB
            m = (dst >= lo) & (dst < hi)
            flat = t_src[m] * NBP + (dst[m] - lo)
            T = np.bincount(flat, weights=coeff[m].astype(np.float64),
                            minlength=tp * NBP)
            blk = np.arange(lo, hi)
            flat_self = types[blk] * NBP + (blk - lo)
            T += np.bincount(flat_self, weights=selfc[blk].astype(np.float64),
                             minlength=tp * NBP)
            T = T.reshape(tp, NBP).astype(np.float32)
            T[nt, 0:NB] = 1.0   # bias row
            Ts.append(T.astype(NP_F8))
            m2 = (src >= lo) & (src < hi)
            flat2 = (src[m2] - lo) * G + g_dst[m2]
            C = np.bincount(flat2, weights=coeff[m2].astype(np.float64),
                            minlength=NBP * G)
            flat2s = (blk - lo) * G + batch[blk]
            C += np.bincount(flat2s, weights=selfc[blk].astype(np.float64),
                             minlength=NBP * G)
            C = C.reshape(NBP, G).astype(np.float32).astype(NP_F8)
            Cs.append(np.ascontiguousarray(C.reshape(cfg.NPAIR, 2, 128, G)))
            cm = np.zeros((1, G), np.float32)
            cm[0, k * GB:(k + 1) * GB] = counts[k * GB:(k + 1) * GB]
            cnts.append(cm)
        return Ts, Cs, cnts

    Ta, Ca, cnta = graph_structs(inputs["ast_edge"], inputs["ast_type"],
                                 inputs["ast_batch"], cfg.TPA, cfg.NTA)
    Tc, Cc, cntc = graph_structs(inputs["cfg_edge"], inputs["cfg_type"],
                                 inputs["cfg_batch"], cfg.TPC, cfg.NTC)

    f32 = lambda x: np.ascontiguousarray(np.asarray(x, np.float32))
    f16 = lambda x: np.ascontiguousarray(np.asarray(x, np.float32).astype(np.float16))
    embT_ast = np.zeros((64, cfg.TPA), np.float32)
    embT_ast[:, 0:cfg.NTA] = f32(inputs["ast_emb"]).T
    embT_cfg = np.zeros((64, cfg.TPC), np.float32)
    embT_cfg[:, 0:cfg.NTC] = f32(inputs["cfg_emb"]).T
    semT = f32(inputs["struct_sem"]).T.copy()  # [SEM, G]

    ln_g = f32(inputs["ln_g"])
    ln_b = f32(inputs["ln_b"])
    Wc = f32(inputs["Wc"])
    Wcp = np.ascontiguousarray(ln_g[:, None] * Wc)
    bcp = (ln_b @ Wc + f32(inputs["bc"])).reshape(2, 1)

    shared = {
        "embT_ast": embT_ast, "embT_cfg": embT_cfg,
        "astW1": f32(inputs["ast_W1"]), "cfgW1": f32(inputs["cfg_W1"]),
        "astb1h": f16(inputs["ast_b1"]).reshape(1, 64),
        "cfgb1h": f16(inputs["cfg_b1"]).reshape(1, 64),
        "astW2h": f16(inputs["ast_W2"]), "cfgW2h": f16(inputs["cfg_W2"]),
        "astb2": f32(inputs["ast_b2"]).reshape(1, 64),
        "cfgb2": f32(inputs["cfg_b2"]).reshape(1, 64),
        "Wg1h": f16(inputs["Wg1"]), "bg1c": f32(inputs["bg1"]).reshape(64, 1),
        "Wsemh": f16(inputs["Wsem"]), "bsemc": f32(inputs["bsem"]).reshape(64, 1),
        "Wg2h": f16(inputs["Wg2"]), "bg2c": f32(inputs["bg2"]).reshape(64, 1),
        "Wcp": Wcp, "bcp": np.ascontiguousarray(bcp),
    }
    in_maps = []
    for k in range(cfg.NCORES):
        m = dict(shared)
        m["T_ast"] = Ta[k]
        m["T_cfg"] = Tc[k]
        m["C_ast"] = Ca[k]
        m["C_cfg"] = Cc[k]
        m["cnt_ast"] = cnta[k]
        m["cnt_cfg"] = cntc[k]
        m["semTh"] = np.ascontiguousarray(
            semT[:, k * GB:(k + 1) * GB].astype(np.float16))
        in_maps.append(m)
    return in_maps


def postprocess(results, cfg: CFG):
    outs = [np.asarray(results[k]["outT"]) for k in range(cfg.NCORES)]
    return np.concatenate(outs, axis=1).T.copy()  # [G, 2]


_CACHED = {}


def kernel(**inputs):
    from concourse.bass_utils import run_bass_kernel_spmd
    cfg = CFG()
    if "nc" not in _CACHED:
        _CACHED["nc"] = build_nc(cfg)
    in_maps = preprocess(inputs, cfg)
    res = run_bass_kernel_spmd(_CACHED["nc"], in_maps,
                               core_ids=list(range(cfg.NCORES)))
    return postprocess(res.results, cfg)
